# revision 2
# baseline (speedup 1.0000x reference)
"""Trainium2 Bass kernel: 3x3 single-channel conv (stride 1, pad 1) on a
4096x4096 fp32 image, sharded over 8 NeuronCores by rows of H.

Numerics: the harness gate is rel_err < 2e-2; pure fp16 (x and w cast to
fp16, fp32 PSUM accumulation, fp16 output) gives ~6e-4, so no hi/lo
split is needed. conv(x, w) is computed on TensorE as 3 accumulating
matmuls per PSUM chunk: S_dj(w) @ x[:, c+dj : c+dj+512] for dj in 0..2,
where S_dj is a banded lhsT [128, 128] encoding the three vertical taps
for horizontal tap dj. This is 3x less PE work and half the HBM traffic
(2 B/px in + 2 B/px out) of the fp32-emulating variant.

Per core (512 output rows): 4 full tiles of 126 rows + an 8-row tail
computed with 8 column-groups stacked in the partition dim (streams
512 cols through the PE instead of 4096).
"""
import sys
sys.path.insert(0, '/opt/trn_rl_repo')
import numpy as np

import concourse.bass as bass
import concourse.mybir as mybir
from concourse.tile import TileContext
from concourse import bass_utils

H = W = 4096
N_CORES = 8
ROWS_PER_CORE = H // N_CORES          # 512
TILE_OUT = 126                        # clean output rows per 128-row tile
CHUNK = 512                           # matmul moving free dim (one PSUM bank)
N_CHUNKS = W // CHUNK                 # 8
FULL_TILES = ROWS_PER_CORE // TILE_OUT        # 4
TAIL_ROWS = ROWS_PER_CORE - FULL_TILES * TILE_OUT   # 8
WPAD = W + 2                          # 4098
TAIL_G = 8                            # tail column groups
TAIL_GW = W // TAIL_G                 # 512
TAIL_K = TAIL_ROWS + 2                # 10 rows per group
TAIL_STACK = TAIL_G * TAIL_K          # 80 partitions
TAIL_M = TAIL_G * TAIL_ROWS           # 64 psum rows

_cache = {}


def _split_multi_waits(nc):
    """This container's walrus accepts only one sync-wait per instruction;
    Tile's tail drain can carry several. Split extras onto NOPs."""
    ctr = 0
    for f in nc.m.functions:
        for bb in f.blocks:
            new_insts = []
            for ins in bb.instructions:
                si = ins.sync_info
                if si is not None and si.on_wait and len(si.on_wait) > 1:
                    waits = list(si.on_wait)
                    for wt in waits[:-1]:
                        ctr += 1
                        new_insts.append(mybir.InstNoOp(
                            name=f"waitfix_{ctr}",
                            sync_info=mybir.SyncInfo(on_wait=[wt], on_update=[]),
                            bass_nofuse=True,
                            engine=ins.engine,
                        ))
                    si.on_wait = [waits[-1]]
                new_insts.append(ins)
            bb.instructions[:] = new_insts
    return nc


def _build_nc(reps=1, mode="full", out_ring="scalar", order="group4",
              xbounds=(0, 514, 2050, WPAD), xbufs=3, osplit=2, hint=True,
              psum_bufs=4, copy_eng="act", unroll=1, obufs=2,
              tail_pos=99, chunk=CHUNK, last_osplit=4):
    f32 = mybir.dt.float32
    f16 = mybir.dt.float16
    do_pe = mode in ("full", "pe_only")
    do_act = mode == "full"
    do_out = mode in ("full", "dma_only")
    nc = bass.Bass()
    xx_d = nc.dram_tensor("xx", [ROWS_PER_CORE + 2, WPAD], f16,
                          kind="ExternalInput")
    # 3 dj blocks, each a banded lhsT [128, 128] (2 zero cols of padding)
    sm_d = nc.dram_tensor("smat", [128, 3 * 128], f16, kind="ExternalInput")
    # tail: 3 dj blocks, stacked block-diag lhsT [80, 64]
    st_d = nc.dram_tensor("stail", [TAIL_STACK, 3 * TAIL_M], f16,
                          kind="ExternalInput")
    bias_in = nc.dram_tensor("bias_in", [128, 1], f32, kind="ExternalInput")
    y = nc.dram_tensor("y", [ROWS_PER_CORE, W], f16, kind="ExternalOutput")

    with TileContext(nc) as tc:
        with tc.tile_pool(name="consts", bufs=1) as cpool, \
             tc.tile_pool(name="xt", bufs=xbufs) as xpool, \
             tc.tile_pool(name="ot", bufs=obufs) as opool, \
             tc.tile_pool(name="psum",
                          bufs=(psum_bufs if chunk == 512 else 3),
                          space="PSUM") as ppool, \
             tc.tile_pool(name="psumt",
                          bufs=(8 - psum_bufs if chunk == 512 else 2),
                          space="PSUM") as ppool_t:
            # const loads ride the SWDGE (gpsimd) ring so they never queue
            # ahead of tile 0's input pieces on the SP HWDGE FIFO
            s_t = cpool.tile([128, 3 * 128], f16)
            nc.gpsimd.dma_start(s_t[:], sm_d[:])
            st_t = cpool.tile([TAIL_STACK, 3 * TAIL_M], f16)
            nc.gpsimd.dma_start(st_t[:], st_d[:])
            b_t = cpool.tile([128, 1], f32)
            nc.gpsimd.dma_start(b_t[:], bias_in[:])
            zt = None
            if mode == "dma_only":
                zt = cpool.tile([128, W], f16)
                nc.gpsimd.memset(zt[:], 0.0)

            out_eng = nc.scalar if out_ring == "scalar" else nc.sync

            def mm_passes(ps_list, xh, lhs_tile, mwidth, chunk_ids, cw=CHUNK):
                """3 accumulating passes (one per horizontal tap dj) over the
                given chunks; pass-outer so consecutive matmuls reuse one
                stationary matrix."""
                if order == "group4":
                    for dj in range(3):
                        scol = dj * mwidth
                        for ci, c0 in enumerate(chunk_ids):
                            nc.tensor.matmul(
                                ps_list[ci],
                                lhs_tile[:, scol:scol + mwidth],
                                xh[:, c0 + dj:c0 + dj + cw],
                                start=(dj == 0), stop=(dj == 2),
                            )
                else:
                    for ci, c0 in enumerate(chunk_ids):
                        for dj in range(3):
                            scol = dj * mwidth
                            nc.tensor.matmul(
                                ps_list[ci],
                                lhs_tile[:, scol:scol + mwidth],
                                xh[:, c0 + dj:c0 + dj + cw],
                                start=(dj == 0), stop=(dj == 2),
                            )

            def full_tile(t):
                k = 128
                r0 = t * TILE_OUT
                xx = xpool.tile([128, WPAD], f16, tag="xx")
                for i in range(len(xbounds) - 1):
                    lo, hi = xbounds[i], xbounds[i + 1]
                    nc.sync.dma_start(xx[:k, lo:hi], xx_d[r0:r0 + k, lo:hi])
                ot = opool.tile([128, W], f16, tag="ot")
                n_chunks = W // chunk
                gsz = max(1, (4 * 512) // chunk)   # chunks per 4-bank group
                for g in range(n_chunks // gsz):
                    chunk_ids = [(g * gsz + i) * chunk for i in range(gsz)]
                    if do_pe:
                        ps_list = []
                        for _ in range(gsz):
                            ps_i = ppool.tile([128, chunk], f32, tag="ps")
                            ps_list.append(ps_i[:, :])
                        mm_passes(ps_list, xx, s_t, 128, chunk_ids, cw=chunk)
                    if do_act:
                        for ci, c0 in enumerate(chunk_ids):
                            if copy_eng == "act":
                                nc.scalar.activation(
                                    ot[:TILE_OUT, c0:c0 + chunk],
                                    ps_list[ci][:TILE_OUT, :],
                                    mybir.ActivationFunctionType.Identity,
                                    bias=b_t[:TILE_OUT, :], scale=1.0,
                                )
                            elif copy_eng == "split":
                                eng = (nc.scalar, nc.vector)[ci % 2]
                                if eng is nc.scalar:
                                    nc.scalar.activation(
                                        ot[:TILE_OUT, c0:c0 + chunk],
                                        ps_list[ci][:TILE_OUT, :],
                                        mybir.ActivationFunctionType.Identity,
                                        bias=b_t[:TILE_OUT, :], scale=1.0,
                                    )
                                else:
                                    nc.vector.tensor_scalar_add(
                                        ot[:TILE_OUT, c0:c0 + chunk],
                                        ps_list[ci][:TILE_OUT, :],
                                        b_t[:TILE_OUT, :])
                            else:
                                nc.vector.tensor_scalar_add(
                                    ot[:TILE_OUT, c0:c0 + chunk],
                                    ps_list[ci][:TILE_OUT, :],
                                    b_t[:TILE_OUT, :])
                if do_out:
                    src_t = ot if do_act else zt
                    osp = last_osplit if t == FULL_TILES - 1 else osplit
                    ow = W // osp
                    for i in range(osp):
                        out_eng.dma_start(
                            y[r0:r0 + TILE_OUT, i * ow:(i + 1) * ow],
                            src_t[:TILE_OUT, i * ow:(i + 1) * ow])

            def tail_load():
                r0 = FULL_TILES * TILE_OUT   # shard row 504
                xxs = xpool.tile([TAIL_STACK, TAIL_GW + 2], f16, tag="txx")
                for g in range(TAIL_G):
                    gc = g * TAIL_GW
                    nc.scalar.dma_start(
                        xxs[g * TAIL_K:(g + 1) * TAIL_K, :],
                        xx_d[r0:r0 + TAIL_K, gc:gc + TAIL_GW + 2])
                return xxs[:, :]

            def tail_tile(xh):
                r0 = FULL_TILES * TILE_OUT   # shard row 504
                ot = opool.tile([TAIL_M, TAIL_GW], f16, tag="tot")
                chunk_ids = [c * CHUNK for c in range(TAIL_GW // CHUNK)]
                if do_pe:
                    ps_list = []
                    for _ in range(len(chunk_ids)):
                        ps_i = ppool_t.tile([TAIL_M, CHUNK], f32, tag="tps")
                        ps_list.append(ps_i[:, :])
                    mm_passes(ps_list, xh, st_t, TAIL_M, chunk_ids,
                              cw=min(CHUNK, TAIL_GW))
                if do_act:
                    for ci, c0 in enumerate(chunk_ids):
                        if copy_eng in ("act", "split"):
                            nc.scalar.activation(
                                ot[:, c0:c0 + CHUNK], ps_list[ci],
                                mybir.ActivationFunctionType.Identity,
                                bias=b_t[:TAIL_M, :], scale=1.0,
                            )
                        else:
                            nc.vector.tensor_scalar_add(
                                ot[:, c0:c0 + CHUNK], ps_list[ci],
                                b_t[:TAIL_M, :])
                if do_out:
                    src_t = ot if do_act else zt
                    for g in range(TAIL_G):
                        out_eng.dma_start(
                            y[r0:r0 + TAIL_ROWS,
                              g * TAIL_GW:(g + 1) * TAIL_GW],
                            src_t[g * TAIL_ROWS:(g + 1) * TAIL_ROWS,
                                  :TAIL_GW])

            def body():
                xs = tail_load()
                if tail_pos == 0:
                    tail_tile(xs)
                for t in range(FULL_TILES):
                    full_tile(t)
                    if t + 1 == tail_pos:
                        tail_tile(xs)
                if tail_pos > FULL_TILES:
                    tail_tile(xs)

            if reps == 1:
                body()
            else:
                hints = (mybir.EngineType.PE,) if hint else ()
                with tc.For_i(0, reps, 1, hint_engines=hints):
                    for _ in range(unroll):
                        body()

    _split_multi_waits(nc)
    return nc


def _make_smat(wh):
    """[128, 3*128] fp16: dj-major blocks, each a banded lhsT [128, 128]
    with band weights w[di, dj]; cols 126, 127 are zero."""
    out = np.zeros((128, 3 * 128), dtype=np.float16)
    idx = np.arange(TILE_OUT)
    for dj in range(3):
        blk = out[:, dj * 128:dj * 128 + 128]
        for di in range(3):
            blk[idx + di, idx] = wh[di, dj]
    return out


def _make_stail(wh):
    """[80, 3*64] fp16: block-diagonal stacked tail lhsT per dj."""
    out = np.zeros((TAIL_STACK, 3 * TAIL_M), dtype=np.float16)
    idx = np.arange(TAIL_ROWS)
    for dj in range(3):
        blk = out[:, dj * TAIL_M:(dj + 1) * TAIL_M]
        for g in range(TAIL_G):
            sub = blk[g * TAIL_K:(g + 1) * TAIL_K,
                      g * TAIL_ROWS:(g + 1) * TAIL_ROWS]
            for di in range(3):
                sub[idx + di, idx] = wh[di, dj]
    return out


def kernel(x, weight, bias):
    x = np.asarray(x, dtype=np.float32)
    weight = np.asarray(weight, dtype=np.float32)
    bias = np.asarray(bias, dtype=np.float32)
    wh = weight.reshape(3, 3).astype(np.float16)

    if "nc" not in _cache:
        _cache["nc"] = _build_nc()
    nc = _cache["nc"]

    xxp = np.zeros((H + 2, WPAD), dtype=np.float16)
    xxp[1:H + 1, 1:W + 1] = x.astype(np.float16)

    smat = _make_smat(wh)
    stail = _make_stail(wh)
    bias_bc = np.full((128, 1), bias[0], dtype=np.float32)

    in_maps = []
    for c in range(N_CORES):
        r0 = c * ROWS_PER_CORE
        in_maps.append({
            "xx": np.ascontiguousarray(xxp[r0:r0 + ROWS_PER_CORE + 2, :]),
            "smat": smat,
            "stail": stail,
            "bias_in": bias_bc,
        })

    _cache["in_maps"] = in_maps
    res = None
    for attempt in range(3):
        try:
            res = bass_utils.run_bass_kernel_spmd(
                nc, in_maps, core_ids=list(range(N_CORES)))
            break
        except Exception:
            if attempt == 2:
                raise
    out = np.empty((H, W), dtype=np.float32)
    for c in range(N_CORES):
        out[c * ROWS_PER_CORE:(c + 1) * ROWS_PER_CORE, :] = (
            res.results[c]["y"].astype(np.float32))
    return out


# revision 10
# speedup vs baseline: 1.2004x; 1.2004x over previous
"""Trainium2 Bass kernel: 3x3 single-channel conv (stride 1, pad 1) on a
4096x4096 fp32 image, sharded over 8 NeuronCores by rows of H.

Numerics: the harness gate is rel_err < 2e-2; pure fp16 (x and w cast to
fp16, fp32 PSUM accumulation, fp16 output) gives ~6e-4, so no hi/lo
split is needed. conv(x, w) is computed on TensorE as 3 accumulating
matmuls per PSUM chunk: S_dj(w) @ x[:, c+dj : c+dj+512] for dj in 0..2,
where S_dj is a banded lhsT [128, 128] encoding the three vertical taps
for horizontal tap dj.

Per core (512 output rows): 4 full tiles of 126 rows + an 8-row tail
computed with 8 column-groups stacked in the partition dim. The tail
stack is packed on host into its own input tensor so it loads as ONE
DMA; each full tile is one input DMA and one output DMA. Outputs ride
the SWDGE (gpsimd) ring so HWDGE only carries the 5 input loads; the
PSUM->SBUF copies alternate between ScalarE and VectorE.
"""
import sys
sys.path.insert(0, '/opt/trn_rl_repo')
import numpy as np

import concourse.bass as bass
import concourse.mybir as mybir
from concourse.tile import TileContext
from concourse import bass_utils

H = W = 4096
N_CORES = 8
ROWS_PER_CORE = H // N_CORES          # 512
TILE_OUT = 126                        # clean output rows per 128-row tile
CHUNK = 512                           # matmul moving free dim (one PSUM bank)
N_CHUNKS = W // CHUNK                 # 8
FULL_TILES = ROWS_PER_CORE // TILE_OUT        # 4
TAIL_ROWS = ROWS_PER_CORE - FULL_TILES * TILE_OUT   # 8
WPAD = W + 2                          # 4098
TAIL_G = 8                            # tail column groups
TAIL_GW = W // TAIL_G                 # 512
TAIL_K = TAIL_ROWS + 2                # 10 rows per group
TAIL_STACK = TAIL_G * TAIL_K          # 80 partitions
TAIL_M = TAIL_G * TAIL_ROWS           # 64 psum rows

_cache = {}


def _split_multi_waits(nc):
    """This container's walrus accepts only one sync-wait per instruction;
    Tile's tail drain can carry several. Split extras onto NOPs."""
    ctr = 0
    for f in nc.m.functions:
        for bb in f.blocks:
            new_insts = []
            for ins in bb.instructions:
                si = ins.sync_info
                if si is not None and si.on_wait and len(si.on_wait) > 1:
                    waits = list(si.on_wait)
                    for wt in waits[:-1]:
                        ctr += 1
                        new_insts.append(mybir.InstNoOp(
                            name=f"waitfix_{ctr}",
                            sync_info=mybir.SyncInfo(on_wait=[wt], on_update=[]),
                            bass_nofuse=True,
                            engine=ins.engine,
                        ))
                    si.on_wait = [waits[-1]]
                new_insts.append(ins)
            bb.instructions[:] = new_insts
    return nc


def _build_nc(reps=1, mode="full", out_ring="scalar", order="group4",
              xbounds=(0, WPAD), xbufs=3, osplit=1, hint=True,
              psum_bufs=4, copy_eng="split", unroll=1, obufs=2,
              tail_pos=99, chunk=CHUNK, last_osplit=1, in_ring="sync",
              flat_reps=1, merge_tail_out=True):
    f32 = mybir.dt.float32
    f16 = mybir.dt.float16
    do_pe = mode in ("full", "pe_only")
    do_act = mode == "full"
    do_out = mode in ("full", "dma_only")
    nc = bass.Bass()
    xx_d = nc.dram_tensor("xx", [ROWS_PER_CORE + 2, WPAD], f16,
                          kind="ExternalInput")
    # host-packed tail stack: partition g*10+k = padded row 504+k,
    # cols g*512 .. g*512+514
    xt_d = nc.dram_tensor("xtail", [TAIL_STACK, TAIL_GW + 2], f16,
                          kind="ExternalInput")
    # 3 dj blocks, each a banded lhsT [128, 128] (2 zero cols of padding)
    sm_d = nc.dram_tensor("smat", [128, 3 * 128], f16, kind="ExternalInput")
    # tail: 3 dj blocks, stacked block-diag lhsT [80, 64]
    st_d = nc.dram_tensor("stail", [TAIL_STACK, 3 * TAIL_M], f16,
                          kind="ExternalInput")
    bias_in = nc.dram_tensor("bias_in", [128, 1], f32, kind="ExternalInput")
    y = nc.dram_tensor("y", [ROWS_PER_CORE, W], f16, kind="ExternalOutput")

    with TileContext(nc) as tc:
        with tc.tile_pool(name="consts", bufs=1) as cpool, \
             tc.tile_pool(name="xt", bufs=xbufs) as xpool, \
             tc.tile_pool(name="ot", bufs=obufs) as opool, \
             tc.tile_pool(name="psum",
                          bufs=(psum_bufs if chunk == 512 else 3),
                          space="PSUM") as ppool, \
             tc.tile_pool(name="psumt",
                          bufs=(8 - psum_bufs if chunk == 512 else 2),
                          space="PSUM") as ppool_t:
            # const loads ride the SWDGE (gpsimd) ring so they never queue
            # ahead of tile 0's input pieces on the SP HWDGE FIFO
            s_t = cpool.tile([128, 3 * 128], f16)
            nc.gpsimd.dma_start(s_t[:], sm_d[:])
            st_t = cpool.tile([TAIL_STACK, 3 * TAIL_M], f16)
            nc.gpsimd.dma_start(st_t[:], st_d[:])
            b_t = cpool.tile([128, 1], f32)
            nc.gpsimd.dma_start(b_t[:], bias_in[:])
            zt = None
            if mode == "dma_only":
                zt = cpool.tile([128, W], f16)
                nc.gpsimd.memset(zt[:], 0.0)

            out_eng = {"scalar": nc.scalar, "sync": nc.sync,
                       "gpsimd": nc.gpsimd}[out_ring]
            in_eng = {"scalar": nc.scalar, "sync": nc.sync,
                      "gpsimd": nc.gpsimd}[in_ring]

            def copy_chunk(dst, src, bias, ci):
                """PSUM -> SBUF fp16 copy (+bias), alternating engines."""
                if copy_eng == "act":
                    e = "a"
                elif copy_eng == "vector":
                    e = "v"
                else:
                    e = ("a", "v")[ci % 2]
                if e == "a":
                    nc.scalar.activation(
                        dst, src, mybir.ActivationFunctionType.Identity,
                        bias=bias, scale=1.0)
                else:
                    nc.vector.tensor_scalar_add(dst, src, bias)

            def mm_passes(ps_list, xh, lhs_tile, mwidth, chunk_ids, cw=CHUNK):
                """3 accumulating passes (one per horizontal tap dj) over the
                given chunks; pass-outer so consecutive matmuls reuse one
                stationary matrix."""
                if order == "group4":
                    for dj in range(3):
                        scol = dj * mwidth
                        for ci, c0 in enumerate(chunk_ids):
                            nc.tensor.matmul(
                                ps_list[ci],
                                lhs_tile[:, scol:scol + mwidth],
                                xh[:, c0 + dj:c0 + dj + cw],
                                start=(dj == 0), stop=(dj == 2),
                            )
                else:
                    for ci, c0 in enumerate(chunk_ids):
                        for dj in range(3):
                            scol = dj * mwidth
                            nc.tensor.matmul(
                                ps_list[ci],
                                lhs_tile[:, scol:scol + mwidth],
                                xh[:, c0 + dj:c0 + dj + cw],
                                start=(dj == 0), stop=(dj == 2),
                            )

            def full_tile(t):
                k = 128
                r0 = t * TILE_OUT
                xx = xpool.tile([128, WPAD], f16, tag="xx")
                for i in range(len(xbounds) - 1):
                    lo, hi = xbounds[i], xbounds[i + 1]
                    in_eng.dma_start(xx[:k, lo:hi], xx_d[r0:r0 + k, lo:hi])
                ot = opool.tile([128, W], f16, tag="ot")
                n_chunks = W // chunk
                gsz = max(1, (4 * 512) // chunk)   # chunks per 4-bank group
                for g in range(n_chunks // gsz):
                    chunk_ids = [(g * gsz + i) * chunk for i in range(gsz)]
                    if do_pe:
                        ps_list = []
                        for _ in range(gsz):
                            ps_i = ppool.tile([128, chunk], f32, tag="ps")
                            ps_list.append(ps_i[:, :])
                        mm_passes(ps_list, xx, s_t, 128, chunk_ids, cw=chunk)
                    if do_act:
                        for ci, c0 in enumerate(chunk_ids):
                            copy_chunk(ot[:TILE_OUT, c0:c0 + chunk],
                                       ps_list[ci][:TILE_OUT, :],
                                       b_t[:TILE_OUT, :], g * gsz + ci)
                if do_out:
                    src_t = ot if do_act else zt
                    osp = last_osplit if t == FULL_TILES - 1 else osplit
                    ow = W // osp
                    for i in range(osp):
                        out_eng.dma_start(
                            y[r0:r0 + TILE_OUT, i * ow:(i + 1) * ow],
                            src_t[:TILE_OUT, i * ow:(i + 1) * ow])

            def tail_load():
                xxs = xpool.tile([TAIL_STACK, TAIL_GW + 2], f16, tag="txx")
                in_eng.dma_start(xxs[:, :], xt_d[:, :])
                return xxs[:, :]

            def tail_tile(xh):
                r0 = FULL_TILES * TILE_OUT   # shard row 504
                ot = opool.tile([TAIL_M, TAIL_GW], f16, tag="tot")
                chunk_ids = [c * CHUNK for c in range(TAIL_GW // CHUNK)]
                if do_pe:
                    ps_list = []
                    for _ in range(len(chunk_ids)):
                        ps_i = ppool_t.tile([TAIL_M, CHUNK], f32, tag="tps")
                        ps_list.append(ps_i[:, :])
                    mm_passes(ps_list, xh, st_t, TAIL_M, chunk_ids,
                              cw=min(CHUNK, TAIL_GW))
                if do_act:
                    for ci, c0 in enumerate(chunk_ids):
                        copy_chunk(ot[:, c0:c0 + CHUNK], ps_list[ci],
                                   b_t[:TAIL_M, :], ci)
                if do_out:
                    src_t = ot if do_act else zt
                    if merge_tail_out:
                        # dst is 3D (g, r, c); src stays a plain 2D SBUF AP
                        # (partition dim must be the single outermost dim) —
                        # flat iteration orders match: p = g*8+r.
                        dst = y[r0:r0 + TAIL_ROWS, :].rearrange(
                            "r (g c) -> g r c", g=TAIL_G)
                        out_eng.dma_start(dst, src_t[:TAIL_M, :TAIL_GW])
                    else:
                        for g in range(TAIL_G):
                            out_eng.dma_start(
                                y[r0:r0 + TAIL_ROWS,
                                  g * TAIL_GW:(g + 1) * TAIL_GW],
                                src_t[g * TAIL_ROWS:(g + 1) * TAIL_ROWS,
                                      :TAIL_GW])

            def body():
                xs = tail_load()
                if tail_pos == 0:
                    tail_tile(xs)
                for t in range(FULL_TILES):
                    full_tile(t)
                    if t + 1 == tail_pos:
                        tail_tile(xs)
                if tail_pos > FULL_TILES:
                    tail_tile(xs)

            if reps == 1:
                for _ in range(flat_reps):
                    body()
            else:
                hints = (mybir.EngineType.PE,) if hint else ()
                with tc.For_i(0, reps, 1, hint_engines=hints):
                    for _ in range(unroll):
                        body()

    _split_multi_waits(nc)
    return nc


def _make_smat(wh):
    """[128, 3*128] fp16: dj-major blocks, each a banded lhsT [128, 128]
    with band weights w[di, dj]; cols 126, 127 are zero."""
    out = np.zeros((128, 3 * 128), dtype=np.float16)
    idx = np.arange(TILE_OUT)
    for dj in range(3):
        blk = out[:, dj * 128:dj * 128 + 128]
        for di in range(3):
            blk[idx + di, idx] = wh[di, dj]
    return out


def _make_stail(wh):
    """[80, 3*64] fp16: block-diagonal stacked tail lhsT per dj."""
    out = np.zeros((TAIL_STACK, 3 * TAIL_M), dtype=np.float16)
    idx = np.arange(TAIL_ROWS)
    for dj in range(3):
        blk = out[:, dj * TAIL_M:(dj + 1) * TAIL_M]
        for g in range(TAIL_G):
            sub = blk[g * TAIL_K:(g + 1) * TAIL_K,
                      g * TAIL_ROWS:(g + 1) * TAIL_ROWS]
            for di in range(3):
                sub[idx + di, idx] = wh[di, dj]
    return out


def kernel(x, weight, bias):
    x = np.asarray(x, dtype=np.float32)
    weight = np.asarray(weight, dtype=np.float32)
    bias = np.asarray(bias, dtype=np.float32)
    wh = weight.reshape(3, 3).astype(np.float16)

    if "nc" not in _cache:
        _cache["nc"] = _build_nc()
    nc = _cache["nc"]

    xxp = np.zeros((H + 2, WPAD), dtype=np.float16)
    xxp[1:H + 1, 1:W + 1] = x.astype(np.float16)

    smat = _make_smat(wh)
    stail = _make_stail(wh)
    bias_bc = np.full((128, 1), bias[0], dtype=np.float32)

    in_maps = []
    for c in range(N_CORES):
        r0 = c * ROWS_PER_CORE
        tr = xxp[r0 + FULL_TILES * TILE_OUT:r0 + FULL_TILES * TILE_OUT
                 + TAIL_K, :]                      # [10, 4098] tail rows
        xtail = np.concatenate(
            [tr[:, g * TAIL_GW:g * TAIL_GW + TAIL_GW + 2]
             for g in range(TAIL_G)], axis=0)      # [80, 514]
        in_maps.append({
            "xx": np.ascontiguousarray(xxp[r0:r0 + ROWS_PER_CORE + 2, :]),
            "xtail": np.ascontiguousarray(xtail),
            "smat": smat,
            "stail": stail,
            "bias_in": bias_bc,
        })

    _cache["in_maps"] = in_maps
    res = None
    for attempt in range(3):
        try:
            res = bass_utils.run_bass_kernel_spmd(
                nc, in_maps, core_ids=list(range(N_CORES)))
            break
        except Exception:
            if attempt == 2:
                raise
    out = np.empty((H, W), dtype=np.float32)
    for c in range(N_CORES):
        out[c * ROWS_PER_CORE:(c + 1) * ROWS_PER_CORE, :] = (
            res.results[c]["y"].astype(np.float32))
    return out


# revision 20
# speedup vs baseline: 1.5109x; 1.2587x over previous
"""Trainium2 Bass kernel: 3x3 single-channel conv (stride 1, pad 1) on a
4096x4096 fp32 image, sharded over 8 NeuronCores by rows of H.

Numerics: the harness gate is rel_err < 2e-2; pure fp16 (x and w cast to
fp16, fp32 PSUM accumulation, fp16 output) gives ~6e-4, so no hi/lo
split is needed. conv(x, w) is computed on TensorE as 3 accumulating
matmuls per PSUM chunk: S_dj(w) @ x[:, c+dj : c+dj+512] for dj in 0..2,
where S_dj is a banded lhsT [128, 128] encoding the three vertical taps
for horizontal tap dj.

Per core (512 output rows): 4 full tiles of 126 rows + an 8-row tail
computed with 8 column-groups stacked in the partition dim. The tail
stack is packed on host into its own input tensor so it loads as ONE
DMA; each full tile is one input DMA (SP HWDGE ring) and one output DMA
(ACT HWDGE ring); the PSUM->SBUF fp16 copies alternate between ScalarE
and VectorE. The timed path unrolls 10 bodies per For_i iteration
(reps stays the total body count) -- the loop boundary otherwise
serializes the pipeline and costs ~14 us/rep.
"""
import sys
sys.path.insert(0, '/opt/trn_rl_repo')
import numpy as np

import concourse.bass as bass
import concourse.mybir as mybir
from concourse.tile import TileContext
from concourse import bass_utils

H = W = 4096
N_CORES = 8
ROWS_PER_CORE = H // N_CORES          # 512
TILE_OUT = 126                        # clean output rows per 128-row tile
CHUNK = 512                           # matmul moving free dim (one PSUM bank)
N_CHUNKS = W // CHUNK                 # 8
FULL_TILES = ROWS_PER_CORE // TILE_OUT        # 4
TAIL_ROWS = ROWS_PER_CORE - FULL_TILES * TILE_OUT   # 8
WPAD = W + 2                          # 4098
TAIL_G = 8                            # tail column groups
TAIL_GW = W // TAIL_G                 # 512
TAIL_K = TAIL_ROWS + 2                # 10 rows per group
TAIL_STACK = TAIL_G * TAIL_K          # 80 partitions
TAIL_M = TAIL_G * TAIL_ROWS           # 64 psum rows

_cache = {}


def _split_multi_waits(nc):
    """This container's walrus accepts only one sync-wait per instruction;
    Tile's tail drain can carry several. Split extras onto NOPs."""
    ctr = 0
    for f in nc.m.functions:
        for bb in f.blocks:
            new_insts = []
            for ins in bb.instructions:
                si = ins.sync_info
                if si is not None and si.on_wait and len(si.on_wait) > 1:
                    waits = list(si.on_wait)
                    for wt in waits[:-1]:
                        ctr += 1
                        new_insts.append(mybir.InstNoOp(
                            name=f"waitfix_{ctr}",
                            sync_info=mybir.SyncInfo(on_wait=[wt], on_update=[]),
                            bass_nofuse=True,
                            engine=ins.engine,
                        ))
                    si.on_wait = [waits[-1]]
                new_insts.append(ins)
            bb.instructions[:] = new_insts
    return nc


def _build_nc(reps=1, mode="full", out_ring="scalar", order="chunk",
              xbounds=(0, WPAD), xbufs=6, osplit=1, hint=True,
              psum_bufs=4, copy_eng="split", unroll=10, obufs=4,
              tail_pos=2, chunk=CHUNK, last_osplit=1, in_ring="sync",
              flat_reps=1, merge_tail_out=True):
    f32 = mybir.dt.float32
    f16 = mybir.dt.float16
    do_pe = mode in ("full", "pe_only", "pe_copy", "pe_dma")
    do_act = mode in ("full", "pe_copy")
    do_out = mode in ("full", "dma_only", "pe_dma", "out_only")
    do_in = mode in ("full", "dma_only", "pe_dma", "in_only")
    nc = bass.Bass()
    xx_d = nc.dram_tensor("xx", [ROWS_PER_CORE + 2, WPAD], f16,
                          kind="ExternalInput")
    # host-packed tail stack: partition g*10+k = padded row 504+k,
    # cols g*512 .. g*512+514
    xt_d = nc.dram_tensor("xtail", [TAIL_STACK, TAIL_GW + 2], f16,
                          kind="ExternalInput")
    # 3 dj blocks, each a banded lhsT [128, 128] (2 zero cols of padding)
    sm_d = nc.dram_tensor("smat", [128, 3 * 128], f16, kind="ExternalInput")
    # tail: 3 dj blocks, stacked block-diag lhsT [80, 64]
    st_d = nc.dram_tensor("stail", [TAIL_STACK, 3 * TAIL_M], f16,
                          kind="ExternalInput")
    bias_in = nc.dram_tensor("bias_in", [128, 1], f32, kind="ExternalInput")
    y = nc.dram_tensor("y", [ROWS_PER_CORE, W], f16, kind="ExternalOutput")

    with TileContext(nc) as tc:
        with tc.tile_pool(name="consts", bufs=1) as cpool, \
             tc.tile_pool(name="xt", bufs=xbufs) as xpool, \
             tc.tile_pool(name="ot", bufs=obufs) as opool, \
             tc.tile_pool(name="psum",
                          bufs=(psum_bufs if chunk == 512 else 3),
                          space="PSUM") as ppool, \
             tc.tile_pool(name="psumt",
                          bufs=(8 - psum_bufs if chunk == 512 else 2),
                          space="PSUM") as ppool_t:
            # const loads ride the SWDGE (gpsimd) ring so they never queue
            # ahead of tile 0's input pieces on the SP HWDGE FIFO
            s_t = cpool.tile([128, 3 * 128], f16)
            nc.gpsimd.dma_start(s_t[:], sm_d[:])
            st_t = cpool.tile([TAIL_STACK, 3 * TAIL_M], f16)
            nc.gpsimd.dma_start(st_t[:], st_d[:])
            b_t = cpool.tile([128, 1], f32)
            nc.gpsimd.dma_start(b_t[:], bias_in[:])
            zt = None
            if do_out and not do_act:
                zt = cpool.tile([128, W], f16)
                nc.gpsimd.memset(zt[:], 0.0)
            cxx = None
            if do_pe and not do_in:
                # isolation modes: matmuls read a static memset tile
                cxx = cpool.tile([128, WPAD], f16)
                nc.gpsimd.memset(cxx[:], 0.0)

            out_eng = {"scalar": nc.scalar, "sync": nc.sync,
                       "gpsimd": nc.gpsimd, "alt": nc.scalar}[out_ring]
            in_eng = {"scalar": nc.scalar, "sync": nc.sync,
                      "gpsimd": nc.gpsimd, "alt": nc.sync}[in_ring]

            def copy_chunk(dst, src, bias, ci):
                """PSUM -> SBUF fp16 copy (+bias), alternating engines."""
                if copy_eng == "act":
                    e = "a"
                elif copy_eng == "vector":
                    e = "v"
                else:
                    e = ("a", "v")[ci % 2]
                if e == "a":
                    nc.scalar.activation(
                        dst, src, mybir.ActivationFunctionType.Identity,
                        bias=bias, scale=1.0)
                else:
                    nc.vector.tensor_scalar_add(dst, src, bias)

            def mm_passes(ps_list, xh, lhs_tile, mwidth, chunk_ids, cw=CHUNK):
                """3 accumulating passes (one per horizontal tap dj) over the
                given chunks; pass-outer so consecutive matmuls reuse one
                stationary matrix."""
                if order == "group4":
                    for dj in range(3):
                        scol = dj * mwidth
                        for ci, c0 in enumerate(chunk_ids):
                            nc.tensor.matmul(
                                ps_list[ci],
                                lhs_tile[:, scol:scol + mwidth],
                                xh[:, c0 + dj:c0 + dj + cw],
                                start=(dj == 0), stop=(dj == 2),
                            )
                else:
                    for ci, c0 in enumerate(chunk_ids):
                        for dj in range(3):
                            scol = dj * mwidth
                            nc.tensor.matmul(
                                ps_list[ci],
                                lhs_tile[:, scol:scol + mwidth],
                                xh[:, c0 + dj:c0 + dj + cw],
                                start=(dj == 0), stop=(dj == 2),
                            )

            def full_tile(t):
                k = 128
                r0 = t * TILE_OUT
                if do_in:
                    xx = xpool.tile([128, WPAD], f16, tag="xx")
                    for i in range(len(xbounds) - 1):
                        lo, hi = xbounds[i], xbounds[i + 1]
                        ie = ((nc.sync, nc.scalar)[i % 2]
                              if in_ring == "alt" else in_eng)
                        ie.dma_start(xx[:k, lo:hi],
                                     xx_d[r0:r0 + k, lo:hi])
                else:
                    xx = cxx
                ot = opool.tile([128, W], f16, tag="ot")
                n_chunks = W // chunk
                gsz = max(1, (4 * 512) // chunk)   # chunks per 4-bank group
                for g in range(n_chunks // gsz):
                    chunk_ids = [(g * gsz + i) * chunk for i in range(gsz)]
                    if do_pe:
                        ps_list = []
                        for _ in range(gsz):
                            ps_i = ppool.tile([128, chunk], f32, tag="ps")
                            ps_list.append(ps_i[:, :])
                        mm_passes(ps_list, xx, s_t, 128, chunk_ids, cw=chunk)
                    if do_act:
                        for ci, c0 in enumerate(chunk_ids):
                            copy_chunk(ot[:TILE_OUT, c0:c0 + chunk],
                                       ps_list[ci][:TILE_OUT, :],
                                       b_t[:TILE_OUT, :], g * gsz + ci)
                if do_out:
                    src_t = ot if do_act else zt
                    osp = last_osplit if t == FULL_TILES - 1 else osplit
                    ow = W // osp
                    for i in range(osp):
                        oe = ((nc.scalar, nc.sync)[i % 2]
                              if out_ring == "alt" else out_eng)
                        oe.dma_start(
                            y[r0:r0 + TILE_OUT, i * ow:(i + 1) * ow],
                            src_t[:TILE_OUT, i * ow:(i + 1) * ow])

            def tail_load():
                if not do_in:
                    return (cxx[:TAIL_STACK, :TAIL_GW + 2]
                            if cxx is not None else None)
                xxs = xpool.tile([TAIL_STACK, TAIL_GW + 2], f16, tag="txx")
                in_eng.dma_start(xxs[:, :], xt_d[:, :])
                return xxs[:, :]

            def tail_tile(xh):
                r0 = FULL_TILES * TILE_OUT   # shard row 504
                ot = opool.tile([TAIL_M, TAIL_GW], f16, tag="tot")
                chunk_ids = [c * CHUNK for c in range(TAIL_GW // CHUNK)]
                if do_pe:
                    ps_list = []
                    for _ in range(len(chunk_ids)):
                        ps_i = ppool_t.tile([TAIL_M, CHUNK], f32, tag="tps")
                        ps_list.append(ps_i[:, :])
                    mm_passes(ps_list, xh, st_t, TAIL_M, chunk_ids,
                              cw=min(CHUNK, TAIL_GW))
                if do_act:
                    for ci, c0 in enumerate(chunk_ids):
                        copy_chunk(ot[:, c0:c0 + CHUNK], ps_list[ci],
                                   b_t[:TAIL_M, :], ci)
                if do_out:
                    src_t = ot if do_act else zt
                    if merge_tail_out:
                        # dst is 3D (g, r, c); src stays a plain 2D SBUF AP
                        # (partition dim must be the single outermost dim) —
                        # flat iteration orders match: p = g*8+r.
                        dst = y[r0:r0 + TAIL_ROWS, :].rearrange(
                            "r (g c) -> g r c", g=TAIL_G)
                        out_eng.dma_start(dst, src_t[:TAIL_M, :TAIL_GW])
                    else:
                        for g in range(TAIL_G):
                            out_eng.dma_start(
                                y[r0:r0 + TAIL_ROWS,
                                  g * TAIL_GW:(g + 1) * TAIL_GW],
                                src_t[g * TAIL_ROWS:(g + 1) * TAIL_ROWS,
                                      :TAIL_GW])

            def body():
                xs = tail_load()
                if tail_pos == 0:
                    tail_tile(xs)
                for t in range(FULL_TILES):
                    full_tile(t)
                    if t + 1 == tail_pos:
                        tail_tile(xs)
                if tail_pos > FULL_TILES:
                    tail_tile(xs)

            if reps == 1:
                for _ in range(flat_reps):
                    body()
            else:
                hints = (mybir.EngineType.PE,) if hint else ()
                u = max(d for d in range(min(unroll, reps), 0, -1)
                        if reps % d == 0)
                with tc.For_i(0, reps // u, 1, hint_engines=hints):
                    for _ in range(u):
                        body()

    _split_multi_waits(nc)
    return nc


def _make_smat(wh):
    """[128, 3*128] fp16: dj-major blocks, each a banded lhsT [128, 128]
    with band weights w[di, dj]; cols 126, 127 are zero."""
    out = np.zeros((128, 3 * 128), dtype=np.float16)
    idx = np.arange(TILE_OUT)
    for dj in range(3):
        blk = out[:, dj * 128:dj * 128 + 128]
        for di in range(3):
            blk[idx + di, idx] = wh[di, dj]
    return out


def _make_stail(wh):
    """[80, 3*64] fp16: block-diagonal stacked tail lhsT per dj."""
    out = np.zeros((TAIL_STACK, 3 * TAIL_M), dtype=np.float16)
    idx = np.arange(TAIL_ROWS)
    for dj in range(3):
        blk = out[:, dj * TAIL_M:(dj + 1) * TAIL_M]
        for g in range(TAIL_G):
            sub = blk[g * TAIL_K:(g + 1) * TAIL_K,
                      g * TAIL_ROWS:(g + 1) * TAIL_ROWS]
            for di in range(3):
                sub[idx + di, idx] = wh[di, dj]
    return out


def kernel(x, weight, bias):
    x = np.asarray(x, dtype=np.float32)
    weight = np.asarray(weight, dtype=np.float32)
    bias = np.asarray(bias, dtype=np.float32)
    wh = weight.reshape(3, 3).astype(np.float16)

    if "nc" not in _cache:
        _cache["nc"] = _build_nc()
    nc = _cache["nc"]

    xxp = np.zeros((H + 2, WPAD), dtype=np.float16)
    xxp[1:H + 1, 1:W + 1] = x.astype(np.float16)

    smat = _make_smat(wh)
    stail = _make_stail(wh)
    bias_bc = np.full((128, 1), bias[0], dtype=np.float32)

    in_maps = []
    for c in range(N_CORES):
        r0 = c * ROWS_PER_CORE
        tr = xxp[r0 + FULL_TILES * TILE_OUT:r0 + FULL_TILES * TILE_OUT
                 + TAIL_K, :]                      # [10, 4098] tail rows
        xtail = np.concatenate(
            [tr[:, g * TAIL_GW:g * TAIL_GW + TAIL_GW + 2]
             for g in range(TAIL_G)], axis=0)      # [80, 514]
        in_maps.append({
            "xx": np.ascontiguousarray(xxp[r0:r0 + ROWS_PER_CORE + 2, :]),
            "xtail": np.ascontiguousarray(xtail),
            "smat": smat,
            "stail": stail,
            "bias_in": bias_bc,
        })

    _cache["in_maps"] = in_maps
    res = None
    for attempt in range(3):
        try:
            res = bass_utils.run_bass_kernel_spmd(
                nc, in_maps, core_ids=list(range(N_CORES)))
            break
        except Exception:
            if attempt == 2:
                raise
    out = np.empty((H, W), dtype=np.float32)
    for c in range(N_CORES):
        out[c * ROWS_PER_CORE:(c + 1) * ROWS_PER_CORE, :] = (
            res.results[c]["y"].astype(np.float32))
    return out
